# revision 1
# baseline (speedup 1.0000x reference)
"""EngramLayer Trainium2 kernel v3 (8-core SPMD, Bass/Tile).

Fully software-pipelined schedule.  Per slot t (one 128-token tile):
  PE : eT-transposes(t+1) | k-quarters(t) | v-quarters(t) | ynT(t-1)
       | one conv-group chunk (conv dt0-7 / dt8-15 / backT pos01 / pos23)
  DVE: eT-copy(t+1) | gate-part2(t-1) | hk(t,q) chase | v-copies(t,q)
       | ynT-copies(t-1) | reduces+gate-part1(t) | y-adds (chunk)
  ACT: kk/vv squares (chase) | tanh(t-1) | gv(t-1) | yn(t-1) | hh(t)
       | silu (chunk)
  Pool: 16 indirect gathers for tile t+2
  SP : loads(t+3), y-stores (chunk)

Conv group g (tiles 4g+1..4g+4) is processed in 4 chunks during slots
4g+5..4g+8, so every slot 5..20 carries exactly one chunk.

PSUM: kq ring2 + vq ring2 + pt ring2 + yc ring2 = 8 banks exactly.
rsqrt via Quake + 1 Newton step; y output bf16 (host upcasts).
"""

import math

import numpy as np
import ml_dtypes

import concourse.bass as bass
import concourse.bacc as bacc
import concourse.mybir as mybir
import concourse.tile as tile
from concourse import bass_utils

F32 = mybir.dt.float32
BF16 = mybir.dt.bfloat16
I32 = mybir.dt.int32
FP8 = mybir.dt.float8e4
AF = mybir.ActivationFunctionType
OP = mybir.AluOpType
AX = mybir.AxisListType

P = 128
B, T, D = 4, 4096, 2048
DM, H, DH = 1024, 16, 64
TABLE = 131072
NCORES = 8
TOK_OUT = (B * T) // NCORES          # 2048 output tokens per core
NT = TOK_OUT // P + 1                # 17 tiles (tile 0 = halo)
NM = DM // P                         # 8 m-blocks
ND = D // P                          # 16 d-tiles
NQ = 4                               # 512-wide d quarters
GRP = 4                              # tiles per conv group
NG = (NT - 1) // GRP                 # 4 conv groups
GW = GRP * P                         # 512 tokens per group
EPS_QK = float(np.finfo(np.float32).eps)
EPS_CONV = 1e-5
KK, DIL = 4, 2
SQD = math.sqrt(D)
SE, SW = 64.0, 32.0   # fp8 scaling for e and key_W
FP8_MODE = False
DMAT_MODE = 1   # backT via xbar DMA transpose; ynT stays on PE
SKK = 1.0 / (SE * SW) ** 2

_CACHE = {}


def build(newton=1, fp8=True, dmat=2, cshift=0):
    nc = bacc.Bacc(None, target_bir_lowering=False)
    ntok = NT * P

    h_in = nc.dram_tensor("h", [ntok, D], BF16, kind="ExternalInput")
    hidx = nc.dram_tensor("hidx", [ntok, H], I32, kind="ExternalInput")
    tbl = nc.dram_tensor("tbl", [H * TABLE, DH], BF16, kind="ExternalInput")
    kwt = nc.dram_tensor("kwt", [NM, P, D], FP8 if fp8 else BF16,
                         kind="ExternalInput")
    vwt = nc.dram_tensor("vwt", [NM, P, D], BF16, kind="ExternalInput")
    cdg = nc.dram_tensor("cdg", [KK * ND, P, P], BF16, kind="ExternalInput")
    idn = nc.dram_tensor("idn", [P, P], BF16, kind="ExternalInput")
    msk = nc.dram_tensor("msk", [P, 1], F32, kind="ExternalInput")
    y_out = nc.dram_tensor("y", [ntok - P, D], BF16, kind="ExternalOutput")

    with tile.TileContext(nc) as tc:
        with (
            tc.tile_pool(name="const", bufs=1) as cp,
            tc.tile_pool(name="io3", bufs=3) as io3,
            tc.tile_pool(name="io2", bufs=2) as io2,
            tc.tile_pool(name="gvp", bufs=6) as gvp,
            tc.tile_pool(name="grp2", bufs=2) as grp2,
            tc.tile_pool(name="grp1", bufs=1) as grp1,
            tc.tile_pool(name="st", bufs=2) as st,
            tc.tile_pool(name="st1", bufs=1) as st1,
            tc.tile_pool(name="pq", bufs=2, space="PSUM") as pq,
            tc.tile_pool(name="pt", bufs=2, space="PSUM") as ptp,
            tc.tile_pool(name="pc", bufs=2, space="PSUM") as pcp,
        ):
            # ---- resident constants ----
            kwt_sb = cp.tile([P, NM, D], FP8 if fp8 else BF16)
            vwt_sb = cp.tile([P, NM, D], BF16)
            for m in range(NM):
                nc.sync.dma_start(kwt_sb[:, m, :], kwt[m])
                nc.sync.dma_start(vwt_sb[:, m, :], vwt[m])
            cdg_sb = cp.tile([P, KK * ND, P], BF16)
            nc.sync.dma_start(cdg_sb[:], cdg[:].rearrange("i p q -> p i q"))
            idn_sb = cp.tile([P, P], BF16)
            nc.sync.dma_start(idn_sb[:], idn[:])
            msk_sb = cp.tile([P, 1], F32)
            nc.sync.dma_start(msk_sb[:], msk[:])

            it_t, h_t, e_t, eT_t = {}, {}, {}, {}
            yn_t, gv_t, late_t, ynTg_t, silu_g = {}, {}, {}, {}, {}

            def load_tile(t):
                if t >= NT:
                    return
                it_t[t] = io3.tile([P, H], I32, tag="idx", name=f"idx{t}")
                nc.sync.dma_start(it_t[t][:], hidx[t * P:(t + 1) * P, :])
                h_t[t] = io2.tile([P, D], BF16, tag="h", name=f"h{t}")
                nc.sync.dma_start(h_t[t][:], h_in[t * P:(t + 1) * P, :])

            def gather_tile(t):
                if t >= NT:
                    return
                e_t[t] = io3.tile([P, DM], BF16, tag="e", name=f"e{t}")
                it_ = it_t.pop(t)
                for hh in range(H):
                    nc.gpsimd.indirect_dma_start(
                        out=e_t[t][:, hh * DH:(hh + 1) * DH],
                        out_offset=None,
                        in_=tbl[:],
                        in_offset=bass.IndirectOffsetOnAxis(
                            ap=it_[:, hh:hh + 1], axis=0),
                    )

            def eT_tile(t):
                """Transpose e(t) -> eT SBUF (xbar DMA or PE+DVE)."""
                if t >= NT:
                    return
                e_sb = e_t.pop(t)
                eT = io2.tile([P, NM, P], BF16, tag="eT", name=f"eT{t}")
                pt_e = ptp.tile([P, NM * P], BF16, tag="pt", name=f"pte{t}")
                for m in range(NM):
                    nc.tensor.transpose(pt_e[:, m * P:(m + 1) * P],
                                        e_sb[:, m * P:(m + 1) * P],
                                        idn_sb[:])
                nc.vector.tensor_copy(
                    eT[:], pt_e[:].rearrange("p (m t) -> p m t", m=NM))
                src = pt_e[:]
                if fp8:
                    eT8 = io2.tile([P, NM, P], FP8, tag="eT8", name=f"eT8_{t}")
                    nc.vector.tensor_scalar(
                        out=eT8[:].rearrange("p m t -> p (m t)"),
                        in0=src, scalar1=SE, scalar2=None, op0=OP.mult)
                else:
                    eT8 = None
                eT_t[t] = (eT, eT8)

            def proj_tile(t):
                """k/v projections with eagerly-consumed psum quarters, then
                reduces + gate chain part 1 (through u)."""
                h_sb = h_t.pop(t)
                eT, eT8 = eT_t.pop(t)

                acc_hk = st.tile([P, NQ], F32, tag="acc_hk", name=f"ahk{t}")
                acc_kk = st.tile([P, NQ], F32, tag="acc_kk", name=f"akk{t}")
                acc_vv = st.tile([P, NQ], F32, tag="acc_vv", name=f"avv{t}")
                acc_hh = st.tile([P, NQ], F32, tag="acc_hh", name=f"ahh{t}")
                scrD = st1.tile([P, 512], BF16, tag="scrD", name=f"sd{t}")
                scrA = st1.tile([P, 512], BF16, tag="scrA", name=f"sa{t}")

                for q in range(NQ):
                    kq = pq.tile([P, 512], F32, tag="kq", name=f"kq{t}_{q}")
                    sl = slice(q * 512, (q + 1) * 512)
                    if fp8:
                        for mp in range(NM // 2):
                            nc.tensor.matmul(
                                kq[:], eT8[:, 2 * mp:2 * mp + 2, :],
                                kwt_sb[:, 2 * mp:2 * mp + 2, sl],
                                start=(mp == 0), stop=(mp == NM // 2 - 1),
                                perf_mode=mybir.MatmulPerfMode.DoubleRow)
                    else:
                        for m in range(NM):
                            nc.tensor.matmul(kq[:], eT[:, m, :],
                                             kwt_sb[:, m, sl],
                                             start=(m == 0),
                                             stop=(m == NM - 1))
                    nc.vector.scalar_tensor_tensor(
                        out=scrD[:], in0=h_sb[:, sl], scalar=1.0, in1=kq[:],
                        op0=OP.mult, op1=OP.mult, accum_out=acc_hk[:, q:q + 1])
                    nc.scalar.activation(scrA[:], kq[:], AF.Square,
                                         accum_out=acc_kk[:, q:q + 1])

                v_sb = io2.tile([P, D], BF16, tag="v", name=f"v{t}")
                for q in range(NQ):
                    vq = pq.tile([P, 512], F32, tag="vq", name=f"vq{t}_{q}")
                    sl = slice(q * 512, (q + 1) * 512)
                    for m in range(NM):
                        nc.tensor.matmul(vq[:], eT[:, m, :],
                                         vwt_sb[:, m, sl],
                                         start=(m == 0), stop=(m == NM - 1))
                    nc.vector.tensor_copy(v_sb[:, sl], vq[:])
                    nc.scalar.activation(scrA[:], vq[:], AF.Square,
                                         accum_out=acc_vv[:, q:q + 1])

                # h^2 stat on DVE (ACT is the tighter engine)
                for q in range(NQ):
                    sl = slice(q * 512, (q + 1) * 512)
                    nc.vector.scalar_tensor_tensor(
                        out=scrD[:], in0=h_sb[:, sl], scalar=1.0,
                        in1=h_sb[:, sl], op0=OP.mult, op1=OP.mult,
                        accum_out=acc_hh[:, q:q + 1])

                # ---- gate chain part 1 (DVE): stats -> u ----
                s_hk = st.tile([P, 1], F32, tag="s_hk", name=f"shk{t}")
                s_kk = st.tile([P, 1], F32, tag="s_kk", name=f"skk{t}")
                s_vv = st.tile([P, 1], F32, tag="s_vv", name=f"svv{t}")
                s_hh = st.tile([P, 1], F32, tag="s_hh", name=f"shh{t}")
                nc.vector.reduce_sum(s_hk[:], acc_hk[:], axis=AX.X)
                nc.vector.reduce_sum(s_kk[:], acc_kk[:], axis=AX.X)
                nc.vector.reduce_sum(s_vv[:], acc_vv[:], axis=AX.X)
                nc.vector.reduce_sum(s_hh[:], acc_hh[:], axis=AX.X)

                de = float(D) * EPS_QK
                t1 = st.tile([P, 1], F32, tag="t1", name=f"t1_{t}")
                pp = st.tile([P, 1], F32, tag="pp", name=f"pp{t}")
                nc.vector.tensor_scalar(out=t1[:], in0=s_kk[:],
                                        scalar1=SKK if fp8 else 1.0,
                                        scalar2=de, op0=OP.mult, op1=OP.add)
                nc.vector.scalar_tensor_tensor(
                    out=pp[:], in0=s_hh[:], scalar=de, in1=t1[:],
                    op0=OP.add, op1=OP.mult)
                r1 = _rsqrt(nc, st, pp[:], f"r1_{t}", newton)
                dot = st.tile([P, 1], F32, tag="dot", name=f"dot{t}")
                nc.vector.scalar_tensor_tensor(
                    out=dot[:], in0=s_hk[:],
                    scalar=SQD / (SE * SW) if fp8 else SQD, in1=r1[:],
                    op0=OP.mult, op1=OP.mult)
                ad = st.tile([P, 1], F32, tag="ad", name=f"ad{t}")
                nc.vector.scalar_tensor_tensor(
                    out=ad[:], in0=dot[:], scalar=-1.0, in1=dot[:],
                    op0=OP.mult, op1=OP.max)
                nc.vector.tensor_scalar(out=ad[:], in0=ad[:], scalar1=1e-6,
                                        scalar2=None, op0=OP.max)
                r2 = _rsqrt(nc, st, ad[:], f"r2_{t}", newton)
                u = st.tile([P, 1], F32, tag="u", name=f"u{t}")
                nc.vector.tensor_tensor(out=u[:], in0=dot[:], in1=r2[:],
                                        op=OP.mult)
                late_t[t] = (u, s_vv, v_sb)

            def late_tile(t):
                """tanh (ACT), gate chain part 2 (DVE), gv + yn (ACT)."""
                u, s_vv, v_sb = late_t.pop(t)
                th = st.tile([P, 1], F32, tag="th", name=f"th{t}")
                nc.scalar.activation(th[:], u[:], AF.Tanh, scale=0.5)
                gate = st.tile([P, 1], F32, tag="gate", name=f"gate{t}")
                nc.vector.tensor_scalar(out=gate[:], in0=th[:], scalar1=0.5,
                                        scalar2=0.5, op0=OP.mult, op1=OP.add)
                if t == 0:
                    nc.vector.tensor_tensor(out=gate[:], in0=gate[:],
                                            in1=msk_sb[:], op=OP.mult)
                gg = st.tile([P, 1], F32, tag="gg", name=f"gg{t}")
                nc.vector.tensor_tensor(out=gg[:], in0=gate[:], in1=gate[:],
                                        op=OP.mult)
                mv = st.tile([P, 1], F32, tag="mv", name=f"mv{t}")
                nc.vector.tensor_scalar(out=mv[:], in0=s_vv[:],
                                        scalar1=1.0 / D, scalar2=None,
                                        op0=OP.mult)
                mc = st.tile([P, 1], F32, tag="mc", name=f"mc{t}")
                nc.vector.tensor_tensor(out=mc[:], in0=gg[:], in1=mv[:],
                                        op=OP.mult)
                nc.vector.tensor_scalar(out=mc[:], in0=mc[:],
                                        scalar1=EPS_CONV, scalar2=None,
                                        op0=OP.add)
                rc = _rsqrt(nc, st, mc[:], f"rc{t}", newton)
                s = st.tile([P, 1], F32, tag="s", name=f"s{t}")
                nc.vector.tensor_tensor(out=s[:], in0=gate[:], in1=rc[:],
                                        op=OP.mult)
                if t > 0:
                    gv = gvp.tile([P, D], BF16, tag="gv", name=f"gv{t}")
                    nc.vector.tensor_scalar(out=gv[:], in0=v_sb[:],
                                            scalar1=gate[:], scalar2=None,
                                            op0=OP.mult)
                    gv_t[t] = gv
                yn = io2.tile([P, D], BF16, tag="yn", name=f"yn{t}")
                nc.scalar.mul(yn[:], v_sb[:], s[:])
                yn_t[t] = yn

            def ynTg_for(g):
                if g not in ynTg_t:
                    ynTg_t[g] = grp2.tile([P, ND, 8 + GW], BF16, tag="ynTg",
                                          name=f"ynTg{g}")
                return ynTg_t[g]

            def ynT_tile(t):
                """Transpose yn(t) into its group buffer (d-major)."""
                yn = yn_t.pop(t)
                if t == 0:
                    buf = ynTg_for(0)
                    pt_h = ptp.tile([P, NM * P], BF16, tag="pt", name="pth0")
                    for dt in range(ND):
                        nc.tensor.transpose(pt_h[:, dt * 8:(dt + 1) * 8],
                                            yn[:, dt * P:(dt + 1) * P],
                                            idn_sb[:, P - 8:P])
                    nc.vector.tensor_copy(
                        buf[:, :, 0:8],
                        pt_h[:, 0:ND * 8].rearrange("p (d t) -> p d t", d=ND))
                    return
                g = (t - 1) // GRP
                buf = ynTg_for(g)
                j = (t - 1) % GRP
                col = 8 + j * P
                if dmat >= 2:
                    # issue from ACT: yn(t-1) is ACT-produced, so this never
                    # blocks the sequencer on a cross-engine wait
                    nc.scalar.dma_start_transpose(buf[:, :, col:col + P], yn[:])
                else:
                    for half in range(2):
                        pt_h = ptp.tile([P, NM * P], BF16, tag="pt",
                                        name=f"pth{t}_{half}")
                        for i in range(8):
                            dt = half * 8 + i
                            nc.tensor.transpose(pt_h[:, i * P:(i + 1) * P],
                                                yn[:, dt * P:(dt + 1) * P],
                                                idn_sb[:])
                        nc.vector.tensor_copy(
                            buf[:, half * 8:(half + 1) * 8, col:col + P],
                            pt_h[:].rearrange("p (d t) -> p d t", d=8))
                if j == GRP - 1 and g + 1 < NG:
                    # carry halo into the next group buffer
                    nxt = ynTg_for(g + 1)
                    nc.vector.tensor_copy(nxt[:, :, 0:8],
                                          buf[:, :, GW:GW + 8])

            def conv_chunk(g, c):
                """Chunk c of conv group g: 0/1 = conv+silu for dt halves,
                2/3 = back-transpose + add + store for position pairs."""
                if g >= NG:
                    return
                if c == 0:
                    shape = [P, GRP, ND * P] if dmat else [P, ND, GW]
                    silu_g[g] = grp1.tile(shape, BF16, tag="silu",
                                          name=f"silu{g}")
                if c in (0, 1):
                    buf = ynTg_t[g]
                    silu_sb = silu_g[g]
                    for dt in range(c * 8, c * 8 + 8):
                        yc = pcp.tile([P, GW], F32, tag="yc",
                                      name=f"yc{g}_{dt}")
                        for k in range(KK):
                            off = 2 + 2 * k
                            nc.tensor.matmul(
                                yc[:],
                                cdg_sb[:, k * ND + dt, :],
                                buf[:, dt, off:off + GW],
                                start=(k == 0), stop=(k == KK - 1))
                        if dmat:
                            # [t-pos, dt] layout so back-transpose is one xbar op
                            nc.scalar.activation(
                                silu_sb[:, :, dt * P:(dt + 1) * P],
                                yc[:], AF.Silu)
                        else:
                            nc.scalar.activation(silu_sb[:, dt, :], yc[:],
                                                 AF.Silu)
                    if c == 1:
                        del ynTg_t[g]
                else:
                    silu_sb = silu_g[g]
                    for j in ((0, 1) if c == 2 else (2, 3)):
                        t = g * GRP + 1 + j
                        gv = gv_t.pop(t)
                        y_sb = io2.tile([P, D], BF16, tag="y", name=f"y{t}")
                        if dmat:
                            ytmp = grp1.tile([P, ND, P], BF16, tag="ytmp",
                                             name=f"yt{t}")
                            nc.scalar.dma_start_transpose(ytmp[:],
                                                          silu_sb[:, j, :])
                            nc.vector.tensor_add(
                                y_sb[:],
                                ytmp[:].rearrange("p a b -> p (a b)"), gv[:])
                        else:
                            for half in range(2):
                                ps = ptp.tile([P, NM * P], BF16, tag="pt",
                                              name=f"ps{t}_{half}")
                                for i in range(8):
                                    dt = half * 8 + i
                                    nc.tensor.transpose(
                                        ps[:, i * P:(i + 1) * P],
                                        silu_sb[:, dt, j * P:(j + 1) * P],
                                        idn_sb[:])
                                sl = slice(half * 1024, (half + 1) * 1024)
                                nc.vector.tensor_add(y_sb[:, sl], ps[:],
                                                     gv[:, sl])
                        nc.sync.dma_start(y_out[(t - 1) * P:t * P, :], y_sb[:])
                    if c == 3:
                        del silu_g[g]

            # ---- pipeline ----
            for t in range(3):
                load_tile(t)
            gather_tile(0)
            gather_tile(1)
            warm = ptp.tile([P, NM * P], BF16, tag="pt", name="warm")
            for i in range(24):
                nc.tensor.transpose(warm[:, (i % 8) * P:(i % 8 + 1) * P],
                                    idn_sb[:], idn_sb[:])
            eT_tile(0)

            NSLOT = NT + 4 + cshift  # main slots + epilogue chunk slots
            for t in range(NSLOT):
                load_tile(t + 3)
                gather_tile(t + 2)
                eT_tile(t + 1)
                if t < NT:
                    proj_tile(t)
                if t >= 1 and (t - 1) in late_t:
                    late_tile(t - 1)
                if t >= 1 and (t - 1) in yn_t:
                    ynT_tile(t - 1)
                if t >= 5 + cshift:
                    conv_chunk((t - 5 - cshift) // GRP, (t - 5 - cshift) % GRP)

    nc.compile()
    return nc


def _rsqrt(nc, pool, x, tag, newton=1):
    """rsqrt on a [128,1] fp32 AP via Quake init + Newton steps."""
    it_ = pool.tile([P, 1], I32, tag="rs_i", name=f"{tag}_i")
    nc.vector.tensor_scalar(out=it_[:], in0=x.bitcast(I32), scalar1=1,
                            scalar2=None, op0=OP.logical_shift_right)
    nc.vector.tensor_scalar(out=it_[:], in0=it_[:], scalar1=-1, scalar2=None,
                            op0=OP.bitwise_xor)
    nc.vector.tensor_scalar(out=it_[:], in0=it_[:], scalar1=0x5F3759DF + 1,
                            scalar2=None, op0=OP.add)
    y = pool.tile([P, 1], F32, tag="rs_y", name=f"{tag}_y")
    t1 = pool.tile([P, 1], F32, tag="rs_t", name=f"{tag}_t")
    src = it_[:].bitcast(F32)
    for _ in range(newton):
        nc.vector.tensor_tensor(out=t1[:], in0=x, in1=src, op=OP.mult)
        nc.vector.tensor_tensor(out=t1[:], in0=t1[:], in1=src, op=OP.mult)
        nc.vector.tensor_scalar(out=t1[:], in0=t1[:], scalar1=-0.5,
                                scalar2=1.5, op0=OP.mult, op1=OP.add)
        nc.vector.tensor_tensor(out=y[:], in0=src, in1=t1[:], op=OP.mult)
        src = y[:]
    return y


def _host_prep(inputs):
    bf = ml_dtypes.bfloat16
    tbl = np.ascontiguousarray(inputs["emb_table"]).astype(bf)
    f8 = ml_dtypes.float8_e4m3
    if FP8_MODE:
        kwt = np.ascontiguousarray(
            np.clip(np.asarray(inputs["key_W"]).T.reshape(NM, P, D) * 32.0,
                    -240.0, 240.0)).astype(f8)
    else:
        kwt = np.ascontiguousarray(
            np.asarray(inputs["key_W"]).T.reshape(NM, P, D)).astype(bf)
    vwt = np.ascontiguousarray(
        np.asarray(inputs["value_W"]).T.reshape(NM, P, D)).astype(bf)
    cw = np.asarray(inputs["conv_w"])  # [D, 1, K]
    cdg = np.zeros((KK * ND, P, P), dtype=bf)
    for k in range(KK):
        for dt in range(ND):
            np.fill_diagonal(cdg[k * ND + dt],
                             cw[dt * P:(dt + 1) * P, 0, k].astype(bf))
    idn = np.eye(P, dtype=bf)
    flat_h = np.asarray(inputs["hidden_states"]).reshape(B * T, D)
    flat_ids = np.asarray(inputs["hash_ids"]).reshape(B * T, H).astype(np.int64)
    flat_ids = (flat_ids + (np.arange(H, dtype=np.int64) * TABLE)[None, :])
    flat_ids = flat_ids.astype(np.int32)
    return tbl, kwt, vwt, cdg, idn, flat_h, flat_ids


def make_in_maps(inputs):
    bf = ml_dtypes.bfloat16
    tbl, kwt, vwt, cdg, idn, flat_h, flat_ids = _host_prep(inputs)
    in_maps = []
    for c in range(NCORES):
        t0 = c * TOK_OUT
        h_c = np.zeros((NT * P, D), dtype=bf)
        ids_c = np.zeros((NT * P, H), dtype=np.int32)
        valid_halo = (t0 % T) != 0
        if valid_halo:
            h_c[:] = flat_h[t0 - P:t0 + TOK_OUT].astype(bf)
            ids_c[:] = flat_ids[t0 - P:t0 + TOK_OUT]
        else:
            h_c[P:] = flat_h[t0:t0 + TOK_OUT].astype(bf)
            ids_c[P:] = flat_ids[t0:t0 + TOK_OUT]
        msk = np.full((P, 1), 1.0 if valid_halo else 0.0, dtype=np.float32)
        in_maps.append(dict(h=h_c, hidx=ids_c, tbl=tbl, kwt=kwt, vwt=vwt,
                            cdg=cdg, idn=idn, msk=msk))
    return in_maps


def kernel(**inputs):
    if "nc" not in _CACHE:
        _CACHE["nc"] = build(fp8=FP8_MODE, dmat=DMAT_MODE)
    nc = _CACHE["nc"]
    in_maps = make_in_maps(inputs)
    res = bass_utils.run_bass_kernel_spmd(nc, in_maps, core_ids=list(range(NCORES)))
    y = np.concatenate([res.results[c]["y"].astype(np.float32)
                        for c in range(NCORES)], axis=0)
    return y.reshape(B, T, D)


if __name__ == "__main__":
    build()
    print("build OK")

